# revision 17
# baseline (speedup 1.0000x reference)
"""GCN-style 8-step SpMM power iteration on 8 Trainium2 NeuronCores.

Math (reference):
    deg = segment_sum(1, col); dis = rsqrt(max(deg,1)) where deg>0 else 0
    norm_e = dis[row_e] * dis[col_e];  row' = row - row.min()
    xX = x @ W_linX + b_linX
    hX_{t+1}[v] = sum_{e: row'_e = v} norm_e * hX_t[col_e] + xX[v]   (8 times)
    out = relu(pp0*xX + pp1*hX_8) @ W_pred + b_pred

Key algebraic trick: norm factorizes per-edge into src/dst node factors, so we
keep the node table pre-scaled: T = dis ⊙ hX. Then one step is
    S[v]   = sum_{e->v} T[col_e]            (pure gather + segment-sum, no
                                             per-edge arithmetic at all)
    hX'[v] = dis_sh[v]*S[v] + xX[v]         (dis_sh = dis shifted by row.min())
    T'[v]  = dis[v]*hX'[v] = (dis*dis_sh)[v]*S[v] + dis[v]*xX[v]

Distribution: nodes dst-sharded over 8 cores.  Each core's 6912 table slots
split in two HALVES (local node id < npc/2 -> half 0); per iteration the
updated shard halves are AllGather'd SEPARATELY (AG#1 fired mid-iteration,
AG#2 at the end) into two rank-major half-tables T0/T1, so the next
iteration's half-0 gathers overlap the tail of the current one.  Gather of
source rows uses dma_gather (int16 indices; edges partitioned by source
half), round-robined over the 4 SWDGE queues so up to 4 Q7 core pairs
generate descriptors concurrently (descriptor generation is the critical
path).  Within each sub-block-half the edges are sorted by source row for
HBM page locality of the 256-B gather reads.  Segment-sum runs on the
TensorEngine: per 32-destination sub-block, 2+2 chunks of 128 edges; each
chunk's 0/1 selection matrix (fp16, host-built) is the stationary operand,
the gathered fp16 messages the moving one, accumulating fp32 in PSUM.
"""

import numpy as np

# problem shape (hardcoded per the task contract)
N = 50000
E = 800000
IN_C = 128
HID = 128
OUT_C = 40
POWER1 = 8

NCORES = 8
SUB_NODES = 32          # destination slots per sub-block (= matmul M)
CHUNK = 128             # edges per chunk (= matmul K)
A_CHUNKS = 2            # chunks per sub-block from source half 0
B_CHUNKS = 2            # chunks per sub-block from source half 1
BATCH_GROUPS = 3        # psum groups (of 4 sub-blocks) per gather batch
GROUP_SUBS = 4          # sub-blocks per psum group ([128,128] psum tile)


# ----------------------------------------------------------------------------
# Host-side preprocessing
# ----------------------------------------------------------------------------

def _pack_core(degA, degB, capA, capB, sub_nodes, target):
    """Greedy best-fit-decreasing packing into `target` bins with per-half
    edge capacities, plus a depth-2 eviction repair for stragglers.
    Returns (list-of-node-lists, overflow-list)."""
    order = np.argsort(-np.maximum(degA, degB), kind="stable")
    bins = [[[], 0, 0] for _ in range(target)]
    overflow = []
    for v in order:
        a, b = int(degA[v]), int(degB[v])
        best, best_slack = -1, None
        for i, (nodes, sa, sb) in enumerate(bins):
            if len(nodes) < sub_nodes and sa + a <= capA and sb + b <= capB:
                slack = (capA - sa - a) + (capB - sb - b)
                if best_slack is None or slack < best_slack:
                    best, best_slack = i, slack
        if best < 0:
            overflow.append(v)
        else:
            bins[best][0].append(v)
            bins[best][1] += a
            bins[best][2] += b

    def try_place(v, depth, forbid):
        a, b = int(degA[v]), int(degB[v])
        for i, (nodes, sa, sb) in enumerate(bins):
            if i in forbid:
                continue
            if len(nodes) < sub_nodes and sa + a <= capA and sb + b <= capB:
                nodes.append(v)
                bins[i][1] += a
                bins[i][2] += b
                return True
        if depth == 0:
            return False
        for i, (nodes, sa, sb) in enumerate(bins):
            if i in forbid:
                continue
            for w in list(nodes):
                aw, bw = int(degA[w]), int(degB[w])
                if sa - aw + a <= capA and sb - bw + b <= capB:
                    nodes.remove(w)
                    bins[i][1] -= aw
                    bins[i][2] -= bw
                    if try_place(w, depth - 1, forbid | {i}):
                        nodes.append(v)
                        bins[i][1] += a
                        bins[i][2] += b
                        return True
                    nodes.append(w)
                    bins[i][1] += aw
                    bins[i][2] += bw
        return False

    still = [v for v in overflow if not try_place(v, 2, frozenset())]
    return [b[0] for b in bins], still


def _preprocess(inputs, n=N, ncores=NCORES):
    x = np.asarray(inputs["x"], dtype=np.float32)
    edge_index = np.asarray(inputs["edge_index"])
    W_linX = np.asarray(inputs["W_linX"], dtype=np.float32)
    b_linX = np.asarray(inputs["b_linX"], dtype=np.float32)
    policy = np.asarray(inputs["policy"], dtype=np.float64)
    W_pred = np.asarray(inputs["W_pred"], dtype=np.float32)
    b_pred = np.asarray(inputs["b_pred"], dtype=np.float32)

    npc = n // ncores
    half_npc = npc // 2
    row = edge_index[0].astype(np.int64)
    col = edge_index[1].astype(np.int64)
    deg = np.bincount(col, minlength=n).astype(np.float64)
    dis = np.where(deg > 0, 1.0 / np.sqrt(np.maximum(deg, 1.0)), 0.0)
    shift = int(row.min())
    dst = row - shift                      # aggregation destination
    # per-dst factor is dis at the *unshifted* row id
    dis_sh = np.zeros(n, dtype=np.float64)
    hi = n - shift
    dis_sh[:hi] = dis[shift:]

    e = np.exp(policy[:2] - policy[:2].max())
    pp = e / e.sum()
    pp0, pp1 = float(pp[0]), float(pp[1])
    b_comb = pp0 + pp1                      # == 1.0, but don't rely on it

    # source-half criterion (fixed a priori; packing honors it for dst too)
    srcH = (col % npc) >= half_npc          # False -> table 0, True -> table 1
    capA, capB = A_CHUNKS * CHUNK, B_CHUNKS * CHUNK

    # pack each (core, half) independently into n_sub_half bins
    for n_sub_half in (108, 120):
        cores = []
        ok = True
        for c in range(ncores):
            m = (dst >= c * npc) & (dst < (c + 1) * npc)
            e_dst = dst[m] - c * npc
            e_src = col[m]
            e_h = srcH[m]
            halves = []
            for h in (0, 1):
                sel = (e_dst >= h * half_npc) & (e_dst < (h + 1) * half_npc)
                d = e_dst[sel] - h * half_npc
                degA = np.bincount(d[~e_h[sel]], minlength=half_npc)
                degB = np.bincount(d[e_h[sel]], minlength=half_npc)
                bins, ov = _pack_core(degA, degB, capA, capB, SUB_NODES,
                                      n_sub_half)
                if ov:
                    ok = False
                halves.append(bins)
            cores.append((e_dst, e_src, e_h, halves))
            if not ok:
                break
        if ok:
            break
    assert ok, "packing failed even at n_sub_half=120"

    n_sub = 2 * n_sub_half
    n_grp = n_sub // GROUP_SUBS
    n_grp_half = n_sub_half // GROUP_SUBS
    assert n_grp % BATCH_GROUPS == 0
    half_slots = n_sub_half * SUB_NODES     # table rows per core per half
    slots = 2 * half_slots
    tabrows = ncores * half_slots           # rows per half-table tensor
    assert tabrows <= 32767, f"half-table {tabrows} exceeds int16 range"

    # slot assignment (half h bins occupy slots [h*half_slots, ...))
    slot_of_node = np.full(n, -1, dtype=np.int64)
    for c, (e_dst, e_src, e_h, halves) in enumerate(cores):
        for h in (0, 1):
            for bi, nodes in enumerate(halves[h]):
                for k, v in enumerate(nodes):
                    slot_of_node[c * npc + h * half_npc + v] = (
                        c * slots + h * half_slots + bi * SUB_NODES + k)
    # an empty slot per half for padding gathers (guaranteed zero row)
    pad_tab = np.zeros(2, dtype=np.int64)
    for h in (0, 1):
        found = False
        for c in range(ncores):
            used = np.zeros(half_slots, dtype=bool)
            lo = slot_of_node[c * npc + h * half_npc:
                              c * npc + (h + 1) * half_npc]
            sl = lo - c * slots - h * half_slots
            used[sl[sl >= 0]] = True
            free = np.flatnonzero(~used)
            if free.size:
                pad_tab[h] = c * half_slots + free[0]
                found = True
                break
        assert found

    # global table row of node v within its half-table
    tab_row = np.full(n, -1, dtype=np.int64)
    cc = np.arange(n) // npc
    s_loc = slot_of_node - cc * slots
    in_h1 = s_loc >= half_slots
    tab_row[~in_h1] = (cc * half_slots + s_loc)[~in_h1]
    tab_row[in_h1] = (cc * half_slots + s_loc - half_slots)[in_h1]
    # consistency: node's slot half == its a-priori source half
    assert np.array_equal(in_h1, (np.arange(n) % npc) >= half_npc)

    per_core = []
    nchA, nchB = n_sub * A_CHUNKS, n_sub * B_CHUNKS
    for c, (e_dst, e_src, e_h, halves) in enumerate(cores):
        loc = slot_of_node[e_dst + c * npc] - c * slots
        dst_bin = loc // SUB_NODES
        dst_k = loc % SUB_NODES
        isA = ~e_h
        S = np.zeros((nchA + nchB, CHUNK, SUB_NODES), dtype=np.float16)
        idxA = np.full(nchA * CHUNK, pad_tab[0], dtype=np.int64)
        idxB = np.full(nchB * CHUNK, pad_tab[1], dtype=np.int64)
        for bi in range(n_sub):
            for half in (True, False):
                sel = (dst_bin == bi) & (isA == half)
                rows = tab_row[e_src[sel]]
                dks = dst_k[sel]
                o = np.argsort(rows, kind="stable")   # HBM page locality
                rows, dks = rows[o], dks[o]
                kk = len(rows)
                cap = capA if half else capB
                assert kk <= cap, (c, bi, half, kk)
                if half:
                    cbase, sbase, idx = bi * A_CHUNKS, 0, idxA
                else:
                    cbase, sbase, idx = bi * B_CHUNKS, nchA, idxB
                for j in range(kk):
                    ch = cbase + j // CHUNK
                    S[sbase + ch, j % CHUNK, dks[j]] = 1.0
                idx[cbase * CHUNK: cbase * CHUNK + kk] = rows
        assert idxA.min() >= 0 and idxA.max() < tabrows
        assert idxB.min() >= 0 and idxB.max() < tabrows

        def wrap_idx(idx):
            # index i consumed from [i % 16, i // 16]; replicate to 128 parts
            w = idx.reshape(-1, 16).T.astype(np.int16)      # [16, n/16]
            return np.tile(w, (8, 1))                        # [128, n/16]

        # slot-layout host arrays
        x_slot = np.zeros((slots, IN_C), dtype=np.float32)
        dis_slot = np.zeros(slots, dtype=np.float64)
        dsh_slot = np.zeros(slots, dtype=np.float64)
        nodes_c = np.arange(c * npc, (c + 1) * npc)
        sl = slot_of_node[nodes_c] - c * slots
        x_slot[sl] = x[nodes_c]
        dis_slot[sl] = dis[nodes_c]
        dsh_slot[sl] = dis_sh[nodes_c]

        grp = lambda v: v.reshape(n_grp, 128).T.astype(np.float32)
        per_core.append({
            "x_slot": x_slot,
            "S": np.ascontiguousarray(
                S.transpose(1, 0, 2).reshape(CHUNK, -1)),    # [128, TC*32]
            "idxA": wrap_idx(idxA),
            "idxB": wrap_idx(idxB),
            "disg": grp(dis_slot),
            "dis2g": grp(dis_slot * dsh_slot),
            "ag": grp((pp1 / b_comb) * dsh_slot),
            "W_linX": W_linX,
            "bX": np.tile(b_linX[None, :], (128, 1)).astype(np.float32),
            "W_pred": (b_comb * W_pred).astype(np.float32),
            "bP": np.tile(b_pred[None, :], (128, 1)).astype(np.float32),
            "ident": np.eye(128, dtype=np.float32),
        })

    meta = dict(n=n, ncores=ncores, npc=npc, n_sub=n_sub, n_grp=n_grp,
                n_grp_half=n_grp_half, slots=slots, half_slots=half_slots,
                tabrows=tabrows, slot_of_node=slot_of_node)
    return meta, per_core


# ----------------------------------------------------------------------------
# Bass program
# ----------------------------------------------------------------------------

def _build_program(meta, iters=POWER1):
    import concourse.bacc as bacc
    import concourse.mybir as mybir
    from concourse import tile

    f32, f16, i16 = mybir.dt.float32, mybir.dt.float16, mybir.dt.int16
    ADD, MULT = mybir.AluOpType.add, mybir.AluOpType.mult

    ncores = meta["ncores"]
    n_sub, n_grp = meta["n_sub"], meta["n_grp"]
    n_grp_half = meta["n_grp_half"]
    slots, half_slots = meta["slots"], meta["half_slots"]
    tabrows = meta["tabrows"]
    n_batches = n_grp // BATCH_GROUPS
    nchA = n_sub * A_CHUNKS
    batch_chunks = BATCH_GROUPS * GROUP_SUBS * A_CHUNKS
    batch_idx = batch_chunks * CHUNK
    TC = n_sub * (A_CHUNKS + B_CHUNKS)

    nc = bacc.Bacc("TRN2", target_bir_lowering=False, debug=False,
                   enable_asserts=False, num_devices=ncores,
                   num_swdge_queues=4)

    x_slot_h = nc.dram_tensor("x_slot", [slots, IN_C], f32, kind="ExternalInput")
    S_h = nc.dram_tensor("S", [CHUNK, TC * SUB_NODES], f16, kind="ExternalInput")
    idxA_h = nc.dram_tensor("idxA", [128, nchA * CHUNK // 16], i16,
                            kind="ExternalInput")
    idxB_h = nc.dram_tensor("idxB", [128, n_sub * B_CHUNKS * CHUNK // 16], i16,
                            kind="ExternalInput")
    disg_h = nc.dram_tensor("disg", [128, n_grp], f32, kind="ExternalInput")
    dis2g_h = nc.dram_tensor("dis2g", [128, n_grp], f32, kind="ExternalInput")
    ag_h = nc.dram_tensor("ag", [128, n_grp], f32, kind="ExternalInput")
    W_h = nc.dram_tensor("W_linX", [IN_C, HID], f32, kind="ExternalInput")
    bX_h = nc.dram_tensor("bX", [128, HID], f32, kind="ExternalInput")
    Wp_h = nc.dram_tensor("W_pred", [HID, OUT_C], f32, kind="ExternalInput")
    bP_h = nc.dram_tensor("bP", [128, OUT_C], f32, kind="ExternalInput")
    id_h = nc.dram_tensor("ident", [128, 128], f32, kind="ExternalInput")

    # two iteration-parity sets x two halves of rank-major gathered tables
    tabs = [[nc.dram_tensor(f"tab{p}{h}", [tabrows, HID], f16,
                            addr_space="Shared") for h in (0, 1)]
            for p in (0, 1)]
    shard1 = nc.dram_tensor("shard1", [half_slots, HID], f16)
    shard2 = nc.dram_tensor("shard2", [half_slots, HID], f16)
    out_h = nc.dram_tensor("out", [slots, OUT_C], f32, kind="ExternalOutput")

    rg = [list(range(ncores))]

    def all_gather(h, p):
        shard = shard1 if h == 0 else shard2
        nc.gpsimd.collective_compute(
            "AllGather", mybir.AluOpType.bypass, replica_groups=rg,
            ins=[shard.ap().opt()], outs=[tabs[p][h].ap().opt()])

    with tile.TileContext(nc, num_cores=ncores) as tc:
        import contextlib
        with contextlib.ExitStack() as ctx:
            cpool = ctx.enter_context(tc.tile_pool(name="const", bufs=1))
            wpool = ctx.enter_context(tc.tile_pool(name="work", bufs=2))
            gpool = ctx.enter_context(tc.tile_pool(name="gatherA", bufs=8))
            hpool = ctx.enter_context(tc.tile_pool(name="gatherB", bufs=8))
            spool = ctx.enter_context(tc.tile_pool(name="stage", bufs=3))
            ppool = ctx.enter_context(
                tc.tile_pool(name="psum", bufs=4, space="PSUM"))
            tpool = ctx.enter_context(
                tc.tile_pool(name="psum2", bufs=2, space="PSUM"))

            # persistent SBUF
            S_sb = cpool.tile([CHUNK, TC * SUB_NODES], f16)
            nc.sync.dma_start(S_sb[:, :], S_h[:, :])
            idxA_sb = cpool.tile([128, nchA * CHUNK // 16], i16)
            nc.sync.dma_start(idxA_sb[:, :], idxA_h[:, :])
            idxB_sb = cpool.tile([128, n_sub * B_CHUNKS * CHUNK // 16], i16)
            nc.sync.dma_start(idxB_sb[:, :], idxB_h[:, :])
            disg = cpool.tile([128, n_grp], f32)
            nc.sync.dma_start(disg[:, :], disg_h[:, :])
            dis2g = cpool.tile([128, n_grp], f32)
            nc.sync.dma_start(dis2g[:, :], dis2g_h[:, :])
            ag = cpool.tile([128, n_grp], f32)
            nc.sync.dma_start(ag[:, :], ag_h[:, :])
            W_sb = cpool.tile([IN_C, HID], f32)
            nc.sync.dma_start(W_sb[:, :], W_h[:, :])
            bX_sb = cpool.tile([128, HID], f32)
            nc.sync.dma_start(bX_sb[:, :], bX_h[:, :])
            Wp_sb = cpool.tile([HID, OUT_C], f32)
            nc.sync.dma_start(Wp_sb[:, :], Wp_h[:, :])
            bP_sb = cpool.tile([128, OUT_C], f32)
            nc.sync.dma_start(bP_sb[:, :], bP_h[:, :])
            ident = cpool.tile([128, 128], f32)
            nc.sync.dma_start(ident[:, :], id_h[:, :])
            xX_sb = cpool.tile([128, n_grp * HID], f16)    # computed below

            def stage_write(g, stage):
                if g < n_grp_half:
                    nc.sync.dma_start(shard1[g * 128:(g + 1) * 128, :],
                                      stage[:, :])
                else:
                    g2 = g - n_grp_half
                    nc.sync.dma_start(shard2[g2 * 128:(g2 + 1) * 128, :],
                                      stage[:, :])

            # ---- prologue: xX = x @ W + b; T0 = dis * xX -> shards -> AGs
            PRO_B = 6                      # groups per batched x load
            for g in range(n_grp):
                gc = slice(g * HID, (g + 1) * HID)
                if g % PRO_B == 0:
                    nb = min(PRO_B, n_grp - g)
                    x_t6 = wpool.tile([128, PRO_B * IN_C], f32, tag="xt")
                    nc.sync.dma_start(
                        x_t6[:, :nb * IN_C].rearrange(
                            "p (j f) -> p j f", f=IN_C),
                        x_slot_h[g * 128:(g + nb) * 128, :].rearrange(
                            "(j p) f -> p j f", p=128))
                tp_ps = tpool.tile([128, 128], f32, tag="tp")
                nc.tensor.transpose(
                    tp_ps[:, :],
                    x_t6[:, (g % PRO_B) * IN_C:(g % PRO_B + 1) * IN_C],
                    ident[:, :])
                xT_sb = wpool.tile([128, 128], f32, tag="xT")
                nc.vector.tensor_copy(xT_sb[:, :], tp_ps[:, :])
                mm_ps = tpool.tile([128, HID], f32, tag="mm2")
                nc.tensor.matmul(mm_ps[:, :], xT_sb[:, :], W_sb[:, :],
                                 start=True, stop=True)
                nc.vector.tensor_tensor(xX_sb[:, gc], mm_ps[:, :],
                                        bX_sb[:, :], op=ADD)
                stage = spool.tile([128, HID], f16, tag="stage")
                nc.vector.tensor_scalar_mul(stage[:, :], xX_sb[:, gc],
                                            disg[:, g:g + 1])
                stage_write(g, stage)
                if g == n_grp_half - 1:
                    all_gather(0, 0)
            all_gather(1, 0)

            # ---- 8 SpMM iterations
            # Gathers round-robin over the 4 SWDGE queues (each queue = its
            # own Q7 core pair); A-half gathers are emitted two batches ahead
            # of B so the previous iteration's AG#2 and the boundary overlap
            # with useful descriptor generation.
            def gather(tin, idx_sb, b, tag, pool, qn):
                m = pool.tile([128, batch_idx], f16, tag=tag)
                nc.gpsimd.dma_gather(
                    m[:, :].rearrange("p (c e) -> p c e", e=HID),
                    tin[0:tabrows, :],
                    idx_sb[:, b * (batch_idx // 16):(b + 1) * (batch_idx // 16)],
                    num_idxs=batch_idx, num_idxs_reg=batch_idx,
                    elem_size=HID, single_packet=False, queue_num=qn)
                return m

            for t in range(iters):
                tin = tabs[t % 2]
                pout = (t + 1) % 2
                last = t == iters - 1
                # queue mixing: A(b) on b%4, B(b) on (b+2)%4, so single-half
                # phases (iteration boundaries) still use all 4 queues
                mAs = {b: gather(tin[0], idxA_sb, b, "mA", gpool, b % 4)
                       for b in (0, 1)}
                if t > 0:
                    all_gather(1, t % 2)      # previous iteration's AG#2
                for b in range(n_batches):
                    mA = mAs.pop(b)
                    mB = gather(tin[1], idxB_sb, b, "mB", hpool, (b + 2) % 4)
                    if b + 2 < n_batches:
                        mAs[b + 2] = gather(tin[0], idxA_sb, b + 2, "mA",
                                            gpool, (b + 2) % 4)
                    if b == n_grp_half // BATCH_GROUPS + 4 and not last:
                        all_gather(0, pout)   # AG#1: shard1 complete by now
                    for u in range(BATCH_GROUPS):
                        g = b * BATCH_GROUPS + u
                        gc = slice(g * HID, (g + 1) * HID)
                        ps = ppool.tile([128, HID], f32, tag="ps")
                        for j in range(GROUP_SUBS):
                            sb = g * GROUP_SUBS + j
                            prange = slice(32 * j, 32 * j + 32)
                            tpos = (0, 32 * j)
                            for k in range(A_CHUNKS):
                                cA = sb * A_CHUNKS + k
                                q = (u * GROUP_SUBS + j) * A_CHUNKS + k
                                nc.tensor.matmul(
                                    ps[prange, :],
                                    S_sb[:, cA * 32:(cA + 1) * 32],
                                    mA[:, q * HID:(q + 1) * HID],
                                    start=(k == 0), stop=False,
                                    tile_position=tpos)
                            for k in range(B_CHUNKS):
                                cB = nchA + sb * B_CHUNKS + k
                                q = (u * GROUP_SUBS + j) * B_CHUNKS + k
                                nc.tensor.matmul(
                                    ps[prange, :],
                                    S_sb[:, cB * 32:(cB + 1) * 32],
                                    mB[:, q * HID:(q + 1) * HID],
                                    start=False, stop=(k == B_CHUNKS - 1),
                                    tile_position=tpos)
                        if not last:
                            t1 = wpool.tile([128, HID], f32, tag="t1")
                            nc.vector.tensor_scalar_mul(
                                t1[:, :], ps[:, :], dis2g[:, g:g + 1])
                            stage = spool.tile([128, HID], f16, tag="stage")
                            nc.vector.scalar_tensor_tensor(
                                stage[:, :], xX_sb[:, gc], disg[:, g:g + 1],
                                t1[:, :], op0=MULT, op1=ADD)
                            stage_write(g, stage)
                        else:
                            u_t = wpool.tile([128, HID], f32, tag="t1")
                            nc.vector.scalar_tensor_tensor(
                                u_t[:, :], ps[:, :], ag[:, g:g + 1],
                                xX_sb[:, gc], op0=MULT, op1=ADD)
                            u_r = wpool.tile([128, HID], f32, tag="t1r")
                            nc.vector.tensor_scalar_max(u_r[:, :], u_t[:, :],
                                                        0.0)
                            tp_ps = tpool.tile([128, 128], f32, tag="tp")
                            nc.tensor.transpose(tp_ps[:, :], u_r[:, :],
                                                ident[:, :])
                            uT_sb = wpool.tile([128, 128], f32, tag="xT")
                            nc.vector.tensor_copy(uT_sb[:, :], tp_ps[:, :])
                            o_ps = tpool.tile([128, OUT_C], f32, tag="mm2")
                            nc.tensor.matmul(o_ps[:, :], uT_sb[:, :],
                                             Wp_sb[:, :], start=True, stop=True)
                            o_sb = spool.tile([128, OUT_C], f32, tag="osb")
                            nc.vector.tensor_tensor(o_sb[:, :], o_ps[:, :],
                                                    bP_sb[:, :], op=ADD)
                            rows = slice(g * 128, (g + 1) * 128)
                            nc.sync.dma_start(out_h[rows, :], o_sb[:, :])

    nc.compile()
    return nc


# ----------------------------------------------------------------------------
# Runner
# ----------------------------------------------------------------------------

def _run(inputs, n=N, ncores=NCORES, trace=False, use_sim=False, iters=POWER1):
    meta, per_core = _preprocess(inputs, n=n, ncores=ncores)
    nc = _build_program(meta, iters=iters)
    in_maps = [dict(pc) for pc in per_core]

    if use_sim:
        from concourse.bass_interp import MultiCoreSim
        sim = MultiCoreSim(nc, num_cores=ncores)
        for c in range(ncores):
            for k, v in in_maps[c].items():
                sim.cores[c].tensor(k)[:] = v
        sim.simulate(check_with_hw=False)
        results = [{"out": np.array(sim.cores[c].tensor("out"))}
                   for c in range(ncores)]
        bres = None
    else:
        from concourse.bass_utils import run_bass_kernel_spmd
        bres = run_bass_kernel_spmd(nc, in_maps, core_ids=list(range(ncores)),
                                    trace=trace)
        results = bres.results

    # unshard: slots -> nodes
    npc, slots = meta["npc"], meta["slots"]
    son = meta["slot_of_node"]
    out = np.zeros((n, OUT_C), dtype=np.float32)
    for c in range(ncores):
        nodes = np.arange(c * npc, (c + 1) * npc)
        out[nodes] = results[c]["out"][son[nodes] - c * slots]
    return out, bres


def kernel(**inputs) -> np.ndarray:
    # Run twice and compare: guards against rare transient device faults
    # (observed once after an unrecoverable-NRT event on a shared terminal).
    out1, _ = _run(inputs)
    out2, _ = _run(inputs)
    if np.allclose(out1, out2, rtol=0, atol=1e-4):
        return out1
    out3, _ = _run(inputs)
    if np.allclose(out1, out3, rtol=0, atol=1e-4):
        return out1
    return out2 if np.allclose(out2, out3, rtol=0, atol=1e-4) else out3
